# revision 28
# baseline (speedup 1.0000x reference)
"""Trainium2 Bass kernel for nn_AttMoE (attention + top-2 MoE routing + pool).

Strategy:
 - Data-parallel over batch: 16 batches -> 8 cores x 2 batches. Zero collectives.
 - Algebraic collapse (exact): out[b] = mean_s(moe_out) @ Wh + bh is linear in
   the expert outputs, so We/Wh fold into wehT = (We @ Wh).T and the dense
   [B,S,E,H] expert compute disappears.  Wo folds into the gating/P matmul
   (Wg3 = Wo @ [Wg | wehT]); the o = attn@Wo projection is never materialized.
   Q/K in-projections fold: Wqi = Wq@Wiq etc.
 - fp32-grade accuracy at fp16 matmul speed: big matmul operands are split
   a = hi + lo (both fp16); a@b = hi@hi + hi@lo + lo@hi accumulated in fp32
   PSUM (dropped lo@lo term is ~2^-22 relative).  This matters because top-2
   gate selection has min |g2-g3| gap ~2e-6 on this data - bf16/tf32 class
   matmuls flip expert assignments and move the output by ~1e-2.  For the
   F=64-contraction projections the hi/lo pieces are stacked along K=128, so
   a full split costs 2 matmuls instead of 3 (and keeps the lo@lo term).
 - Attention per (batch, head): scores in [q,k] layout; exp on ACT with fused
   1/sqrt(DH) scale and accum_out denominator (scores are in [-8,8]: no max
   subtraction); softmax normalization fused into the fp16 hi/lo split of the
   attention weights; [q,k]->[k,q] transpose done by the DMA engines (xbar
   2-byte transpose), not the PE.  AV then yields attn_out^T directly in the
   layout the gating matmul needs as lhsT.
"""
import sys
import numpy as np

for _p in ("/opt/trn_rl_repo",):
    if _p not in sys.path:
        sys.path.insert(0, _p)

B, S, F, H, NH, E = 16, 1024, 64, 512, 4, 12
DH = H // NH          # 128
NCORES = 8
BPC = B // NCORES     # batches per core = 2
T = BPC * S           # tokens per core = 2048
NBH = BPC * NH        # (batch, head) pairs per core = 8
NTT = T // 128        # token tiles per core = 16
TPB = S // 128        # token tiles per batch = 8
E2 = 2 * E            # gating matmul width: [logits | P]

_PROGRAM = None       # cached compiled Bacc program


def _split16(a):
    hi = a.astype(np.float16)
    lo = (a - hi.astype(np.float32)).astype(np.float16)
    return hi, lo


def host_prep(inputs):
    """Fold weights on host (numpy).  Returns per-core input maps."""
    f32 = np.float32
    g = {k: np.asarray(v, dtype=f32) for k, v in inputs.items()}

    Wqi = (g["Wq"] @ g["Wiq"]).astype(f32)
    Wki = (g["Wk"] @ g["Wik"]).astype(f32)
    Wvi = (g["Wv"] @ g["Wiv"]).astype(f32)
    bqi = (g["bq"] @ g["Wiq"] + g["biq"]).astype(f32)
    bki = (g["bk"] @ g["Wik"] + g["bik"]).astype(f32)
    bvi = (g["bv"] @ g["Wiv"] + g["biv"]).astype(f32)

    weh = (g["We"] @ g["Wh"]).squeeze(-1)            # [E, H]
    wg2 = np.concatenate([g["Wg"], weh.T], axis=1)   # [H, 2E]
    wg3 = (g["Wo"] @ wg2).astype(f32)                # [H, 2E]
    beh = (g["be"] @ g["Wh"]).squeeze(-1)            # [E]
    b3 = np.concatenate([g["bo"] @ g["Wg"], g["bo"] @ weh.T + beh]).astype(f32)

    # per-(head,dh) biases for the transposed q/k layout: [128, 8]
    bqk = np.stack(
        [bqi[h * DH:(h + 1) * DH] for h in range(NH)]
        + [bki[h * DH:(h + 1) * DH] for h in range(NH)], axis=1).astype(f32)

    shared = {"bqk": np.ascontiguousarray(bqk),
              "bvB": np.ascontiguousarray(np.broadcast_to(bvi, (128, H))),
              "wg3": np.ascontiguousarray(wg3),
              "b3B": np.ascontiguousarray(np.broadcast_to(b3, (128, E2))),
              "bhS": np.asarray(g["bh"], dtype=f32).reshape(1, 1)}
    for nm, wgt in (("wq", Wqi), ("wk", Wki), ("wv", Wvi)):
        hi, lo = _split16(wgt)
        shared[nm + "h"] = np.ascontiguousarray(hi)
        shared[nm + "l"] = np.ascontiguousarray(lo)

    x = g["x"]
    in_maps = []
    for c in range(NCORES):
        xT = np.concatenate(
            [np.ascontiguousarray(x[c * BPC + j].T) for j in range(BPC)], axis=1)
        hi, lo = _split16(xT)
        m = dict(shared)
        m["xh"] = np.ascontiguousarray(hi)
        m["xl"] = np.ascontiguousarray(lo)
        in_maps.append(m)
    return in_maps


def build_program():
    """Build + compile the (SPMD, identical-per-core) Bacc program."""
    global _PROGRAM
    if _PROGRAM is not None:
        return _PROGRAM

    import concourse.bacc as bacc
    import concourse.tile as tile
    from concourse import mybir

    DT = mybir.dt.float32
    F16 = mybir.dt.float16
    AX = mybir.AxisListType
    AF = mybir.ActivationFunctionType
    OP = mybir.AluOpType

    nc = bacc.Bacc(trn_type="TRN2", target_bir_lowering=False, debug=False)

    d_xh = nc.dram_tensor("xh", [F, T], F16, kind="ExternalInput").ap()
    d_xl = nc.dram_tensor("xl", [F, T], F16, kind="ExternalInput").ap()
    dw = {}
    for nm in ("wqh", "wql", "wkh", "wkl", "wvh", "wvl"):
        dw[nm] = nc.dram_tensor(nm, [F, H], F16, kind="ExternalInput").ap()
    d_bqk = nc.dram_tensor("bqk", [DH, 2 * NH], DT, kind="ExternalInput").ap()
    d_bvB = nc.dram_tensor("bvB", [128, H], DT, kind="ExternalInput").ap()
    d_wg3 = nc.dram_tensor("wg3", [H, E2], DT, kind="ExternalInput").ap()
    d_b3B = nc.dram_tensor("b3B", [128, E2], DT, kind="ExternalInput").ap()
    d_bhS = nc.dram_tensor("bhS", [1, 1], DT, kind="ExternalInput").ap()
    d_out = nc.dram_tensor("out", [1, BPC], DT, kind="ExternalOutput").ap()

    ISQ = float(1.0 / np.sqrt(DH))

    from contextlib import ExitStack
    with tile.TileContext(nc) as tc, ExitStack() as ctx:
        cst = ctx.enter_context(tc.tile_pool(name="cst", bufs=1))
        big = ctx.enter_context(tc.tile_pool(name="big", bufs=1))
        v32p = ctx.enter_context(tc.tile_pool(name="v32p", bufs=2))
        qkp = ctx.enter_context(tc.tile_pool(name="qkp", bufs=2))
        attnp = ctx.enter_context(tc.tile_pool(name="attnp", bufs=4))
        wsp = ctx.enter_context(tc.tile_pool(name="wsp", bufs=2))
        wtp = ctx.enter_context(tc.tile_pool(name="wtp", bufs=2))
        smal = ctx.enter_context(tc.tile_pool(name="smal", bufs=12))
        ps_proj = ctx.enter_context(tc.tile_pool(name="ps_proj", bufs=2, space="PSUM"))
        ps_sc = ctx.enter_context(tc.tile_pool(name="ps_sc", bufs=3, space="PSUM"))
        ps_av = ctx.enter_context(tc.tile_pool(name="ps_av", bufs=3, space="PSUM"))

        # ---- constants / weights ----
        xh = cst.tile([F, T], F16); nc.sync.dma_start(xh[:], d_xh)
        xl = cst.tile([F, T], F16); nc.sync.dma_start(xl[:], d_xl)
        w = {}
        for nm in ("wqh", "wql", "wkh", "wkl", "wvh", "wvl"):
            w[nm] = cst.tile([F, H], F16, tag=nm, name=nm)
            nc.sync.dma_start(w[nm][:], dw[nm])
        bqk = cst.tile([DH, 2 * NH], DT); nc.sync.dma_start(bqk[:], d_bqk)
        bvB = cst.tile([128, H], DT); nc.sync.dma_start(bvB[:], d_bvB)
        wg3 = cst.tile([128, H // 128, E2], DT)
        nc.sync.dma_start(wg3[:], d_wg3.rearrange("(c p) n -> p c n", p=128))
        b3B = cst.tile([128, E2], DT); nc.sync.dma_start(b3B[:], d_b3B)
        bhS = cst.tile([1, 1], DT); nc.sync.dma_start(bhS[:], d_bhS)
        ones = cst.tile([128, 1], DT)
        nc.vector.memset(ones[:], 1.0)

        # ---- V projection (K-stacked split), split into fp16 hi/lo ----
        vhi = big.tile([128, NTT, H], F16)
        vlo = big.tile([128, NTT, H], F16)
        for t in range(NTT):
            xhs = xh[:, t * 128:(t + 1) * 128]
            xls = xl[:, t * 128:(t + 1) * 128]
            pv = ps_proj.tile([128, H], DT, tag="ps_proj")
            nc.tensor.matmul(pv[:], xhs, w["wvh"][:], start=True, stop=False)
            nc.tensor.matmul(pv[:], xhs, w["wvl"][:], start=False, stop=False)
            nc.tensor.matmul(pv[:], xls, w["wvh"][:], start=False, stop=True)
            v32 = v32p.tile([128, H], DT, tag="v32")
            nc.vector.scalar_tensor_tensor(
                v32[:], pv[:], 1.0, bvB[:], op0=OP.mult, op1=OP.add)
            nc.vector.tensor_copy(vhi[:, t, :], v32[:])
            nc.vector.tensor_tensor(vlo[:, t, :], v32[:], vhi[:, t, :],
                                    op=OP.subtract)

        o_normT = big.tile([128, NBH, S], DT)   # normalized attn-out^T (fp32)

        # ---- attention per (batch, head), software-pipelined:
        # qkT of head i+1 is emitted before AV of head i so the PE has
        # independent work while head i's attention weights stream through
        # DVE and the DMA transposes.
        bh_list = [(b, h) for b in range(BPC) for h in range(NH)]

        def emit_qkT(b, h):
            qhi = qkp.tile([DH, S], F16, tag="qhi", name="qhi")
            qlo = qkp.tile([DH, S], F16, tag="qlo", name="qlo")
            khi = qkp.tile([DH, S], F16, tag="khi", name="khi")
            klo = qkp.tile([DH, S], F16, tag="klo", name="klo")
            hs = slice(h * DH, (h + 1) * DH)
            for wh_, wl_, dsth, dstl, bcol in (
                    (w["wqh"], w["wql"], qhi, qlo, h),
                    (w["wkh"], w["wkl"], khi, klo, NH + h)):
                for sp in range(2):
                    xsp = slice(b * S + sp * 512, b * S + (sp + 1) * 512)
                    pp = ps_proj.tile([128, 512], DT, tag="ps_proj",
                                      name="pp")
                    nc.tensor.matmul(pp[:], wh_[:, hs], xh[:, xsp],
                                     start=True, stop=False)
                    nc.tensor.matmul(pp[:], wh_[:, hs], xl[:, xsp],
                                     start=False, stop=False)
                    nc.tensor.matmul(pp[:], wl_[:, hs], xh[:, xsp],
                                     start=False, stop=True)
                    dsp = slice(sp * 512, (sp + 1) * 512)
                    nc.scalar.activation(dsth[:, dsp], pp[:], AF.Identity,
                                         bias=bqk[:, bcol:bcol + 1])
                    nc.vector.scalar_tensor_tensor(
                        dstl[:, dsp], pp[:], bqk[:, bcol:bcol + 1],
                        dsth[:, dsp], op0=OP.add, op1=OP.subtract)
            return qhi, qlo, khi, klo

        def emit_av(b, h, whiT, wloT):
            bh = b * NH + h
            hs = slice(h * DH, (h + 1) * DH)
            for sp in range(2):
                qsp = slice(sp * 512, (sp + 1) * 512)
                pav = ps_av.tile([128, 512], DT, tag="ps_av", name="pav")
                for kc in range(TPB):
                    vh = vhi[:, b * TPB + kc, hs]
                    vl = vlo[:, b * TPB + kc, hs]
                    nc.tensor.matmul(pav[:], vh, whiT[:, kc, qsp],
                                     start=(kc == 0), stop=False)
                    nc.tensor.matmul(pav[:], vh, wloT[:, kc, qsp],
                                     start=False, stop=False)
                    nc.tensor.matmul(pav[:], vl, whiT[:, kc, qsp],
                                     start=False, stop=(kc == TPB - 1))
                nc.vector.tensor_copy(o_normT[:, bh, qsp], pav[:])

        pending_av = None
        tiles = emit_qkT(*bh_list[0])
        for i, (b, h) in enumerate(bh_list):
            bh = b * NH + h
            qhi, qlo, khi, klo = tiles
            hs = slice(h * DH, (h + 1) * DH)

            whiT = wtp.tile([128, TPB, S], F16, tag="whiT")
            wloT = wtp.tile([128, TPB, S], F16, tag="wloT")
            for qc in range(TPB):
                qs = slice(qc * 128, (qc + 1) * 128)
                at = attnp.tile([128, S], DT, tag="attn")
                denh = smal.tile([128, 2], DT, tag="denh")
                for half in range(2):
                    ks = slice(half * 512, (half + 1) * 512)
                    psc = ps_sc.tile([128, 512], DT, tag="ps_sc")
                    nc.tensor.matmul(psc[:], qhi[:, qs], khi[:, ks],
                                     start=True, stop=False)
                    nc.tensor.matmul(psc[:], qhi[:, qs], klo[:, ks],
                                     start=False, stop=False)
                    nc.tensor.matmul(psc[:], qlo[:, qs], khi[:, ks],
                                     start=False, stop=True)
                    nc.scalar.activation(
                        at[:, ks], psc[:], AF.Exp, scale=ISQ,
                        accum_out=denh[:, half:half + 1])
                den = smal.tile([128, 1], DT, tag="den")
                nc.vector.tensor_tensor(
                    den[:], denh[:, 0:1], denh[:, 1:2], op=OP.add)
                rec = smal.tile([128, 1], DT, tag="rec")
                nc.vector.reciprocal(rec[:], den[:])
                whi_t = wsp.tile([128, S], F16, tag="whi")
                nc.vector.tensor_scalar_mul(whi_t[:], at[:], rec[:])
                nc.sync.dma_start_transpose(whiT[:, :, qs], whi_t[:])
                wlo_t = wsp.tile([128, S], F16, tag="wlo")
                nc.vector.scalar_tensor_tensor(
                    wlo_t[:], at[:], rec[:], whi_t[:],
                    op0=OP.mult, op1=OP.subtract)
                nc.sync.dma_start_transpose(wloT[:, :, qs], wlo_t[:])

            if i + 1 < len(bh_list):
                tiles = emit_qkT(*bh_list[i + 1])

            if pending_av is not None:
                emit_av(*pending_av)
            pending_av = (b, h, whiT, wloT)

        if pending_av is not None:
            emit_av(*pending_av)

        # ---- gating + P matmul: G = o_normT.T @ Wg3 (+ b3) ----
        Gall = big.tile([128, NTT, E2], DT)
        for t in range(NTT):
            b, tt = divmod(t, TPB)
            pg = ps_proj.tile([128, E2], DT, tag="ps_proj")
            for hc in range(NH):
                nc.tensor.matmul(
                    pg[:],
                    o_normT[:, b * NH + hc, tt * 128:(tt + 1) * 128],
                    wg3[:, hc, :],
                    start=(hc == 0), stop=(hc == NH - 1))
            nc.vector.scalar_tensor_tensor(
                Gall[:, t, :], pg[:], 1.0, b3B[:], op0=OP.mult, op1=OP.add)

        # ---- top-2 softmax gating, combine with P, pool ----
        L = Gall[:, :, 0:E]
        P = Gall[:, :, E:E2]
        t1 = big.tile([128, NTT], DT)
        nc.vector.reduce_max(t1[:], L, axis=AX.X)
        m1 = big.tile([128, NTT, E], DT)
        nc.vector.tensor_tensor(
            m1[:], L, t1[:].unsqueeze(2).broadcast_to([128, NTT, E]),
            op=OP.is_ge)
        L2 = big.tile([128, NTT, E], DT)
        nc.vector.scalar_tensor_tensor(
            L2[:], m1[:], -1e30, L, op0=OP.mult, op1=OP.add)
        t2 = big.tile([128, NTT], DT)
        nc.vector.reduce_max(t2[:], L2[:], axis=AX.X)
        msk = big.tile([128, NTT, E], DT)
        nc.vector.tensor_tensor(
            msk[:], L, t2[:].unsqueeze(2).broadcast_to([128, NTT, E]),
            op=OP.is_ge)
        EL = big.tile([128, NTT, E], DT)
        nc.scalar.activation(EL[:], L, AF.Exp)
        Em = big.tile([128, NTT, E], DT)
        nc.vector.tensor_tensor(Em[:], msk[:], EL[:], op=OP.mult)
        EmP = big.tile([128, NTT, E], DT)
        nc.vector.tensor_tensor(EmP[:], Em[:], P, op=OP.mult)
        num = big.tile([128, NTT], DT)
        nc.vector.reduce_sum(num[:], EmP[:], axis=AX.X)
        den2 = big.tile([128, NTT], DT)
        nc.vector.reduce_sum(den2[:], Em[:], axis=AX.X)
        r2 = big.tile([128, NTT], DT)
        nc.vector.reciprocal(r2[:], den2[:])
        tokv = big.tile([128, NTT], DT)
        nc.vector.tensor_tensor(tokv[:], num[:], r2[:], op=OP.mult)
        bsum = big.tile([128, BPC], DT)
        nc.vector.reduce_sum(
            bsum[:], tokv[:].rearrange("p (b t) -> p b t", b=BPC), axis=AX.X)
        pfin = ps_proj.tile([1, BPC], DT, tag="ps_proj")
        nc.tensor.matmul(pfin[:], ones[:], bsum[:], start=True, stop=True)
        osb = smal.tile([1, BPC], DT, tag="osb")
        nc.scalar.activation(osb[:], pfin[:], AF.Identity,
                             bias=bhS[:], scale=float(1.0 / S))
        nc.sync.dma_start(d_out, osb[:])

    nc.compile()
    _PROGRAM = nc
    return nc


def run_cores(in_maps, trace=False, **kw):
    from concourse.bass_utils import run_bass_kernel_spmd
    nc = build_program()
    return run_bass_kernel_spmd(nc, in_maps, list(range(NCORES)),
                                trace=trace, **kw)


def kernel(**inputs) -> np.ndarray:
    in_maps = host_prep(inputs)
    res = run_cores(in_maps, trace=False)
    out = np.concatenate([res.results[c]["out"][0] for c in range(NCORES)])
    return out.astype(np.float32)


# revision 29
# speedup vs baseline: 1.1885x; 1.1885x over previous
"""Trainium2 Bass kernel for nn_AttMoE (attention + top-2 MoE routing + pool).

Strategy:
 - Data-parallel over batch: 16 batches -> 8 cores x 2 batches. Zero collectives.
 - Algebraic collapse (exact): out[b] = mean_s(moe_out) @ Wh + bh is linear in
   the expert outputs, so We/Wh fold into wehT = (We @ Wh).T and the dense
   [B,S,E,H] expert compute disappears.  Wo folds into the gating/P matmul
   (Wg3 = Wo @ [Wg | wehT]); the o = attn@Wo projection is never materialized.
   Q/K in-projections fold: Wqi = Wq@Wiq etc.
 - fp32-grade accuracy at fp16 matmul speed: big matmul operands are split
   a = hi + lo (both fp16); a@b = hi@hi + hi@lo + lo@hi accumulated in fp32
   PSUM (dropped lo@lo term is ~2^-22 relative).  This matters because top-2
   gate selection has min |g2-g3| gap ~2e-6 on this data - bf16/tf32 class
   matmuls flip expert assignments and move the output by ~1e-2.  For the
   F=64-contraction projections the hi/lo pieces are stacked along K=128, so
   a full split costs 2 matmuls instead of 3 (and keeps the lo@lo term).
 - Attention per (batch, head): scores in [q,k] layout; exp on ACT with fused
   1/sqrt(DH) scale and accum_out denominator (scores are in [-8,8]: no max
   subtraction); softmax normalization fused into the fp16 hi/lo split of the
   attention weights; [q,k]->[k,q] transpose done by the DMA engines (xbar
   2-byte transpose), not the PE.  AV then yields attn_out^T directly in the
   layout the gating matmul needs as lhsT.
"""
import sys
import numpy as np

for _p in ("/opt/trn_rl_repo",):
    if _p not in sys.path:
        sys.path.insert(0, _p)

B, S, F, H, NH, E = 16, 1024, 64, 512, 4, 12
DH = H // NH          # 128
NCORES = 8
BPC = B // NCORES     # batches per core = 2
T = BPC * S           # tokens per core = 2048
NBH = BPC * NH        # (batch, head) pairs per core = 8
NTT = T // 128        # token tiles per core = 16
TPB = S // 128        # token tiles per batch = 8
E2 = 2 * E            # gating matmul width: [logits | P]

_PROGRAM = None       # cached compiled Bacc program


def _split16(a):
    hi = a.astype(np.float16)
    lo = (a - hi.astype(np.float32)).astype(np.float16)
    return hi, lo


def host_prep(inputs):
    """Fold weights on host (numpy).  Returns per-core input maps."""
    f32 = np.float32
    g = {k: np.asarray(v, dtype=f32) for k, v in inputs.items()}

    Wqi = (g["Wq"] @ g["Wiq"]).astype(f32)
    Wki = (g["Wk"] @ g["Wik"]).astype(f32)
    Wvi = (g["Wv"] @ g["Wiv"]).astype(f32)
    bqi = (g["bq"] @ g["Wiq"] + g["biq"]).astype(f32)
    bki = (g["bk"] @ g["Wik"] + g["bik"]).astype(f32)
    bvi = (g["bv"] @ g["Wiv"] + g["biv"]).astype(f32)

    weh = (g["We"] @ g["Wh"]).squeeze(-1)            # [E, H]
    wg2 = np.concatenate([g["Wg"], weh.T], axis=1)   # [H, 2E]
    wg3 = (g["Wo"] @ wg2).astype(f32)                # [H, 2E]
    beh = (g["be"] @ g["Wh"]).squeeze(-1)            # [E]
    b3 = np.concatenate([g["bo"] @ g["Wg"], g["bo"] @ weh.T + beh]).astype(f32)

    # per-(head,dh) biases for the transposed q/k layout: [128, 8]
    bqk = np.stack(
        [bqi[h * DH:(h + 1) * DH] for h in range(NH)]
        + [bki[h * DH:(h + 1) * DH] for h in range(NH)], axis=1).astype(f32)

    shared = {"bqk": np.ascontiguousarray(bqk),
              "bvB": np.ascontiguousarray(np.broadcast_to(bvi, (128, H))),
              "wg3": np.ascontiguousarray(wg3),
              "b3B": np.ascontiguousarray(np.broadcast_to(b3, (128, E2))),
              "bhS": np.asarray(g["bh"], dtype=f32).reshape(1, 1)}
    for nm, wgt in (("wq", Wqi), ("wk", Wki), ("wv", Wvi)):
        hi, lo = _split16(wgt)
        shared[nm + "h"] = np.ascontiguousarray(hi)
        shared[nm + "l"] = np.ascontiguousarray(lo)

    x = g["x"]
    in_maps = []
    for c in range(NCORES):
        xT = np.concatenate(
            [np.ascontiguousarray(x[c * BPC + j].T) for j in range(BPC)], axis=1)
        hi, lo = _split16(xT)
        m = dict(shared)
        m["xh"] = np.ascontiguousarray(hi)
        m["xl"] = np.ascontiguousarray(lo)
        in_maps.append(m)
    return in_maps


def build_program():
    """Build + compile the (SPMD, identical-per-core) Bacc program."""
    global _PROGRAM
    if _PROGRAM is not None:
        return _PROGRAM

    import concourse.bacc as bacc
    import concourse.tile as tile
    from concourse import mybir

    DT = mybir.dt.float32
    F16 = mybir.dt.float16
    AX = mybir.AxisListType
    AF = mybir.ActivationFunctionType
    OP = mybir.AluOpType

    nc = bacc.Bacc(trn_type="TRN2", target_bir_lowering=False, debug=False)

    d_xh = nc.dram_tensor("xh", [F, T], F16, kind="ExternalInput").ap()
    d_xl = nc.dram_tensor("xl", [F, T], F16, kind="ExternalInput").ap()
    dw = {}
    for nm in ("wqh", "wql", "wkh", "wkl", "wvh", "wvl"):
        dw[nm] = nc.dram_tensor(nm, [F, H], F16, kind="ExternalInput").ap()
    d_bqk = nc.dram_tensor("bqk", [DH, 2 * NH], DT, kind="ExternalInput").ap()
    d_bvB = nc.dram_tensor("bvB", [128, H], DT, kind="ExternalInput").ap()
    d_wg3 = nc.dram_tensor("wg3", [H, E2], DT, kind="ExternalInput").ap()
    d_b3B = nc.dram_tensor("b3B", [128, E2], DT, kind="ExternalInput").ap()
    d_bhS = nc.dram_tensor("bhS", [1, 1], DT, kind="ExternalInput").ap()
    d_out = nc.dram_tensor("out", [1, BPC], DT, kind="ExternalOutput").ap()

    ISQ = float(1.0 / np.sqrt(DH))

    from contextlib import ExitStack
    with tile.TileContext(nc) as tc, ExitStack() as ctx:
        cst = ctx.enter_context(tc.tile_pool(name="cst", bufs=1))
        big = ctx.enter_context(tc.tile_pool(name="big", bufs=1))
        v32p = ctx.enter_context(tc.tile_pool(name="v32p", bufs=2))
        qkp = ctx.enter_context(tc.tile_pool(name="qkp", bufs=2))
        attnp = ctx.enter_context(tc.tile_pool(name="attnp", bufs=4))
        wsp = ctx.enter_context(tc.tile_pool(name="wsp", bufs=2))
        wtp = ctx.enter_context(tc.tile_pool(name="wtp", bufs=2))
        smal = ctx.enter_context(tc.tile_pool(name="smal", bufs=12))
        ps_proj = ctx.enter_context(tc.tile_pool(name="ps_proj", bufs=2, space="PSUM"))
        ps_sc = ctx.enter_context(tc.tile_pool(name="ps_sc", bufs=3, space="PSUM"))
        ps_av = ctx.enter_context(tc.tile_pool(name="ps_av", bufs=3, space="PSUM"))

        # ---- constants / weights ----
        xh = cst.tile([F, T], F16); nc.sync.dma_start(xh[:], d_xh)
        xl = cst.tile([F, T], F16); nc.sync.dma_start(xl[:], d_xl)
        w = {}
        for nm in ("wqh", "wql", "wkh", "wkl", "wvh", "wvl"):
            w[nm] = cst.tile([F, H], F16, tag=nm, name=nm)
            nc.sync.dma_start(w[nm][:], dw[nm])
        bqk = cst.tile([DH, 2 * NH], DT); nc.sync.dma_start(bqk[:], d_bqk)
        bvB = cst.tile([128, H], DT); nc.sync.dma_start(bvB[:], d_bvB)
        wg3 = cst.tile([128, H // 128, E2], DT)
        nc.sync.dma_start(wg3[:], d_wg3.rearrange("(c p) n -> p c n", p=128))
        b3B = cst.tile([128, E2], DT); nc.sync.dma_start(b3B[:], d_b3B)
        bhS = cst.tile([1, 1], DT); nc.sync.dma_start(bhS[:], d_bhS)
        ones = cst.tile([128, 1], DT)
        nc.vector.memset(ones[:], 1.0)

        # ---- V projection (K-stacked split), split into fp16 hi/lo ----
        vhi = big.tile([128, NTT, H], F16)
        vlo = big.tile([128, NTT, H], F16)
        for t in range(NTT):
            xhs = xh[:, t * 128:(t + 1) * 128]
            xls = xl[:, t * 128:(t + 1) * 128]
            pv = ps_proj.tile([128, H], DT, tag="ps_proj")
            nc.tensor.matmul(pv[:], xhs, w["wvh"][:], start=True, stop=False)
            nc.tensor.matmul(pv[:], xhs, w["wvl"][:], start=False, stop=False)
            nc.tensor.matmul(pv[:], xls, w["wvh"][:], start=False, stop=True)
            v32 = v32p.tile([128, H], DT, tag="v32")
            nc.vector.scalar_tensor_tensor(
                v32[:], pv[:], 1.0, bvB[:], op0=OP.mult, op1=OP.add)
            nc.vector.tensor_copy(vhi[:, t, :], v32[:])
            nc.vector.tensor_tensor(vlo[:, t, :], v32[:], vhi[:, t, :],
                                    op=OP.subtract)

        o_normT = big.tile([128, NBH, S], DT)   # normalized attn-out^T (fp32)

        # ---- attention per (batch, head), software-pipelined:
        # qkT of head i+1 is emitted before AV of head i so the PE has
        # independent work while head i's attention weights stream through
        # DVE and the DMA transposes.
        bh_list = [(b, h) for b in range(BPC) for h in range(NH)]

        def emit_qkT(b, h):
            qhi = qkp.tile([DH, S], F16, tag="qhi", name="qhi")
            qlo = qkp.tile([DH, S], F16, tag="qlo", name="qlo")
            khi = qkp.tile([DH, S], F16, tag="khi", name="khi")
            klo = qkp.tile([DH, S], F16, tag="klo", name="klo")
            hs = slice(h * DH, (h + 1) * DH)
            for wh_, wl_, dsth, dstl, bcol in (
                    (w["wqh"], w["wql"], qhi, qlo, h),
                    (w["wkh"], w["wkl"], khi, klo, NH + h)):
                for sp in range(2):
                    xsp = slice(b * S + sp * 512, b * S + (sp + 1) * 512)
                    pp = ps_proj.tile([128, 512], DT, tag="ps_proj",
                                      name="pp")
                    nc.tensor.matmul(pp[:], wh_[:, hs], xh[:, xsp],
                                     start=True, stop=False)
                    nc.tensor.matmul(pp[:], wh_[:, hs], xl[:, xsp],
                                     start=False, stop=False)
                    nc.tensor.matmul(pp[:], wl_[:, hs], xh[:, xsp],
                                     start=False, stop=True)
                    dsp = slice(sp * 512, (sp + 1) * 512)
                    nc.scalar.activation(dsth[:, dsp], pp[:], AF.Identity,
                                         bias=bqk[:, bcol:bcol + 1])
                    nc.vector.scalar_tensor_tensor(
                        dstl[:, dsp], pp[:], bqk[:, bcol:bcol + 1],
                        dsth[:, dsp], op0=OP.add, op1=OP.subtract)
            return qhi, qlo, khi, klo

        tiles = emit_qkT(*bh_list[0])
        for i, (b, h) in enumerate(bh_list):
            bh = b * NH + h
            qhi, qlo, khi, klo = tiles
            hs = slice(h * DH, (h + 1) * DH)

            whiT = wtp.tile([128, TPB, S], F16, tag="whiT")
            wloT = wtp.tile([128, TPB, S], F16, tag="wloT")
            for qc in range(TPB):
                qs = slice(qc * 128, (qc + 1) * 128)
                at = attnp.tile([128, S], DT, tag="attn")
                denh = smal.tile([128, 2], DT, tag="denh")
                for half in range(2):
                    ks = slice(half * 512, (half + 1) * 512)
                    psc = ps_sc.tile([128, 512], DT, tag="ps_sc")
                    nc.tensor.matmul(psc[:], qhi[:, qs], khi[:, ks],
                                     start=True, stop=False)
                    nc.tensor.matmul(psc[:], qhi[:, qs], klo[:, ks],
                                     start=False, stop=False)
                    nc.tensor.matmul(psc[:], qlo[:, qs], khi[:, ks],
                                     start=False, stop=True)
                    nc.scalar.activation(
                        at[:, ks], psc[:], AF.Exp, scale=ISQ,
                        accum_out=denh[:, half:half + 1])
                den = smal.tile([128, 1], DT, tag="den")
                nc.vector.tensor_tensor(
                    den[:], denh[:, 0:1], denh[:, 1:2], op=OP.add)
                rec = smal.tile([128, 1], DT, tag="rec")
                nc.vector.reciprocal(rec[:], den[:])
                whi_t = wsp.tile([128, S], F16, tag="whi")
                nc.vector.tensor_scalar_mul(whi_t[:], at[:], rec[:])
                nc.sync.dma_start_transpose(whiT[:, :, qs], whi_t[:])
                wlo_t = wsp.tile([128, S], F16, tag="wlo")
                nc.vector.scalar_tensor_tensor(
                    wlo_t[:], at[:], rec[:], whi_t[:],
                    op0=OP.mult, op1=OP.subtract)
                nc.sync.dma_start_transpose(wloT[:, :, qs], wlo_t[:])

            if i + 1 < len(bh_list):
                tiles = emit_qkT(*bh_list[i + 1])

            for sp in range(2):
                qsp = slice(sp * 512, (sp + 1) * 512)
                pav = ps_av.tile([128, 512], DT, tag="ps_av")
                for kc in range(TPB):
                    vh = vhi[:, b * TPB + kc, hs]
                    vl = vlo[:, b * TPB + kc, hs]
                    nc.tensor.matmul(pav[:], vh, whiT[:, kc, qsp],
                                     start=(kc == 0), stop=False)
                    nc.tensor.matmul(pav[:], vh, wloT[:, kc, qsp],
                                     start=False, stop=False)
                    nc.tensor.matmul(pav[:], vl, whiT[:, kc, qsp],
                                     start=False, stop=(kc == TPB - 1))
                nc.vector.tensor_copy(o_normT[:, bh, qsp], pav[:])

        # ---- gating + P matmul: G = o_normT.T @ Wg3 (+ b3) ----
        Gall = big.tile([128, NTT, E2], DT)
        for t in range(NTT):
            b, tt = divmod(t, TPB)
            pg = ps_proj.tile([128, E2], DT, tag="ps_proj")
            for hc in range(NH):
                nc.tensor.matmul(
                    pg[:],
                    o_normT[:, b * NH + hc, tt * 128:(tt + 1) * 128],
                    wg3[:, hc, :],
                    start=(hc == 0), stop=(hc == NH - 1))
            nc.vector.scalar_tensor_tensor(
                Gall[:, t, :], pg[:], 1.0, b3B[:], op0=OP.mult, op1=OP.add)

        # ---- top-2 softmax gating, combine with P, pool ----
        L = Gall[:, :, 0:E]
        P = Gall[:, :, E:E2]
        t1 = big.tile([128, NTT], DT)
        nc.vector.reduce_max(t1[:], L, axis=AX.X)
        m1 = big.tile([128, NTT, E], DT)
        nc.vector.tensor_tensor(
            m1[:], L, t1[:].unsqueeze(2).broadcast_to([128, NTT, E]),
            op=OP.is_ge)
        L2 = big.tile([128, NTT, E], DT)
        nc.vector.scalar_tensor_tensor(
            L2[:], m1[:], -1e30, L, op0=OP.mult, op1=OP.add)
        t2 = big.tile([128, NTT], DT)
        nc.vector.reduce_max(t2[:], L2[:], axis=AX.X)
        msk = big.tile([128, NTT, E], DT)
        nc.vector.tensor_tensor(
            msk[:], L, t2[:].unsqueeze(2).broadcast_to([128, NTT, E]),
            op=OP.is_ge)
        EL = big.tile([128, NTT, E], DT)
        nc.scalar.activation(EL[:], L, AF.Exp)
        Em = big.tile([128, NTT, E], DT)
        nc.vector.tensor_tensor(Em[:], msk[:], EL[:], op=OP.mult)
        EmP = big.tile([128, NTT, E], DT)
        nc.vector.tensor_tensor(EmP[:], Em[:], P, op=OP.mult)
        num = big.tile([128, NTT], DT)
        nc.vector.reduce_sum(num[:], EmP[:], axis=AX.X)
        den2 = big.tile([128, NTT], DT)
        nc.vector.reduce_sum(den2[:], Em[:], axis=AX.X)
        r2 = big.tile([128, NTT], DT)
        nc.vector.reciprocal(r2[:], den2[:])
        tokv = big.tile([128, NTT], DT)
        nc.vector.tensor_tensor(tokv[:], num[:], r2[:], op=OP.mult)
        bsum = big.tile([128, BPC], DT)
        nc.vector.reduce_sum(
            bsum[:], tokv[:].rearrange("p (b t) -> p b t", b=BPC), axis=AX.X)
        pfin = ps_proj.tile([1, BPC], DT, tag="ps_proj")
        nc.tensor.matmul(pfin[:], ones[:], bsum[:], start=True, stop=True)
        osb = smal.tile([1, BPC], DT, tag="osb")
        nc.scalar.activation(osb[:], pfin[:], AF.Identity,
                             bias=bhS[:], scale=float(1.0 / S))
        nc.sync.dma_start(d_out, osb[:])

    nc.compile()
    _PROGRAM = nc
    return nc


def run_cores(in_maps, trace=False, **kw):
    from concourse.bass_utils import run_bass_kernel_spmd
    nc = build_program()
    return run_bass_kernel_spmd(nc, in_maps, list(range(NCORES)),
                                trace=trace, **kw)


def kernel(**inputs) -> np.ndarray:
    in_maps = host_prep(inputs)
    res = run_cores(in_maps, trace=False)
    out = np.concatenate([res.results[c]["out"][0] for c in range(NCORES)])
    return out.astype(np.float32)
